# revision 1
# baseline (speedup 1.0000x reference)
"""GCN-LSTM Trainium kernel: host preprocessing + Bass/Tile program builder.

Strategy (8-core SPMD, node-parallel):
 - Nodes sharded 12500/core (padded to 12800 = 25 groups of 512 = 100 blocks
   of 128). All LSTM state/IO for a core's nodes is local.
 - Edges assigned to the core owning their dst. Per dst-block of 128 nodes,
   edges are packed into CPB chunks of 128 (E_blk = CPB*128 >= max block
   degree sum, global). Mean aggregation per block:
       one-hot M[lane, slot] = (slot_col[lane] == iota) built on DVE,
       psum[slot, 0:128]  += M.T @ G        (G = gathered src feature rows)
       psum[slot, 128]    += M.T @ ones     (degree)
   then h_agg = psum[:, :128] * 1/max(deg,1), PE-transposed to feature-major.
 - Gather of G via gpsimd.dma_gather (bulk SWDGE gather, int16 idxs):
   per-core table of the distinct src rows it references, split in two
   32768-row windows (blocks 0-49 -> window 0, 50-99 -> window 1) so every
   block's edges index one window with int16.
 - LSTM (3 layers, seq_len=1) per 512-node group, feature-major:
   gates[g,n] = sum_k W.T[k,g] x[k,n] accumulated in PSUM [128,512],
   sigmoid/tanh with fused per-partition bias on ACT, cell math on DVE.
 - h_n/c_n written feature-major; host transposes back.
"""
import numpy as np
import concourse.bacc as bacc
import concourse.bass as bass
import concourse.mybir as mybir
import concourse.tile as tile
from concourse.masks import make_identity

P = 128
D = 128          # feature dim
NL = 3           # LSTM layers
WIN = 32768      # gather-table window rows (int16 addressable)
f32 = mybir.dt.float32
i16 = mybir.dt.int16


# ---------------------------------------------------------------- host side
def preprocess(feature, src, dst, n_cores=8):
    """Build per-core data + global structure params."""
    N = feature.shape[0]
    assert N % n_cores == 0
    np_real = N // n_cores                     # 12500
    ngrp = -(-np_real // 512)                  # 25 groups of 512
    np_pad = ngrp * 512                        # 12800
    nblk = np_pad // P                         # 100
    half_blks = nblk // 2                      # 50

    src = np.asarray(src, np.int64)
    dst = np.asarray(dst, np.int64)
    core_of = dst // np_real

    per_core = []
    maxcnt = 0
    for m in range(n_cores):
        sel = core_of == m
        s_m = src[sel]
        d_m = dst[sel] - m * np_real
        order = np.argsort(d_m, kind="stable")
        s_m = s_m[order]
        d_m = d_m[order]
        blk = d_m >> 7
        cnt = np.bincount(blk, minlength=nblk)
        maxcnt = max(maxcnt, int(cnt.max()))
        per_core.append((s_m, d_m, blk, cnt))

    cpb = -(-max(maxcnt, 1) // P)              # chunks per block
    e_blk = cpb * P
    ncol = nblk * cpb
    nslot = ncol * P

    cores = []
    for m in range(n_cores):
        s_m, d_m, blk, cnt = per_core[m]
        offs = np.zeros(nblk + 1, np.int64)
        np.cumsum(cnt, out=offs[1:])
        k = np.arange(len(d_m)) - offs[blk]
        j = (blk * cpb + (k >> 7)) * P + (k & 127)   # slot id, col-major

        gidx = np.zeros(nslot, np.int16)
        slot = np.full(nslot, 300.0, np.float32)
        slot[j] = (d_m & 127).astype(np.float32)

        tabs = []
        lo = blk < half_blks
        for h, selh in enumerate((lo, ~lo)):
            u = np.unique(s_m[selh])
            assert len(u) <= WIN, f"core {m} half {h}: {len(u)} distinct srcs > {WIN}"
            t = np.zeros((WIN, D), np.float32)
            t[: len(u)] = feature[u]
            tabs.append(t)
            gidx[j[selh]] = np.searchsorted(u, s_m[selh]).astype(np.int16)

        idxw = np.tile(gidx.reshape(-1, 16).T, (8, 1))        # [128, nslot//16]
        slot_cols = np.ascontiguousarray(slot.reshape(ncol, P).T)  # [128, ncol]
        deg = np.bincount(d_m, minlength=np_pad).astype(np.float32)
        recip = (1.0 / np.maximum(deg, 1.0)).astype(np.float32)
        recip_cols = np.ascontiguousarray(recip.reshape(nblk, P).T)  # [128, nblk]
        cores.append(dict(tab0=tabs[0], tab1=tabs[1], recips=recip_cols,
                          idxs=np.ascontiguousarray(idxw), slots=slot_cols))

    meta = dict(np_real=np_real, np_pad=np_pad, nblk=nblk, cpb=cpb,
                ngrp=ngrp, half_blks=half_blks, ncol=ncol, nslot=nslot)
    return cores, meta


def node_arrays(feature, h0, c0, n_cores, np_real, np_pad):
    """Per-core feature-major node tensors."""
    out = []
    for m in range(n_cores):
        sl = slice(m * np_real, (m + 1) * np_real)
        ft = np.zeros((P, np_pad), np.float32)
        ft[:, :np_real] = feature[sl].T
        h0t = np.zeros((NL, P, np_pad), np.float32)
        c0t = np.zeros((NL, P, np_pad), np.float32)
        for l in range(NL):
            h0t[l, :, :np_real] = h0[l, sl].T
            c0t[l, :, :np_real] = c0[l, sl].T
        out.append(dict(featT=ft, h0T=h0t, c0T=c0t))
    return out


def weight_arrays(w_ih0, w_hh0, w_ih_rest, w_hh_rest, b_ih, b_hh):
    w = dict(
        wih0a=np.ascontiguousarray(w_ih0.T[0:128]),    # [128, 512]
        wih0b=np.ascontiguousarray(w_ih0.T[128:256]),
        whh0=np.ascontiguousarray(w_hh0.T),            # [128, 512]
        wih1=np.ascontiguousarray(w_ih_rest[0].T),
        whh1=np.ascontiguousarray(w_hh_rest[0].T),
        wih2=np.ascontiguousarray(w_ih_rest[1].T),
        whh2=np.ascontiguousarray(w_hh_rest[1].T),
    )
    bsum = b_ih + b_hh                                  # [3, 512]
    bias = np.zeros((P, NL * 4), np.float32)
    for l in range(NL):
        for g in range(4):
            bias[:, l * 4 + g] = bsum[l, g * 128:(g + 1) * 128]
    w["biases"] = bias
    return {k: np.ascontiguousarray(v.astype(np.float32)) for k, v in w.items()}


# ---------------------------------------------------------------- device side
def build_program(meta, f32r=True, nq=4, scratch=32768):
    nblk, cpb, ngrp, half_blks = (meta["nblk"], meta["cpb"], meta["ngrp"],
                                  meta["half_blks"])
    ncol, nslot, np_pad = meta["ncol"], meta["nslot"], meta["np_pad"]
    bpg = nblk // ngrp                   # blocks per group (4)
    assert bpg * ngrp == nblk

    mdt = mybir.dt.float32r if f32r else f32

    nc = bacc.Bacc(None, target_bir_lowering=False, debug=False,
                   num_swdge_queues=nq, dynamic_dma_scratch_size=scratch)
    tab0 = nc.dram_tensor("tab0", [WIN, D], mdt, kind="ExternalInput")
    tab1 = nc.dram_tensor("tab1", [WIN, D], mdt, kind="ExternalInput")
    idxs = nc.dram_tensor("idxs", [P, nslot // 16], i16, kind="ExternalInput")
    slots = nc.dram_tensor("slots", [P, ncol], f32, kind="ExternalInput")
    recips = nc.dram_tensor("recips", [P, nblk], f32, kind="ExternalInput")
    featT = nc.dram_tensor("featT", [P, np_pad], mdt, kind="ExternalInput")
    h0T = nc.dram_tensor("h0T", [NL, P, np_pad], mdt, kind="ExternalInput")
    c0T = nc.dram_tensor("c0T", [NL, P, np_pad], f32, kind="ExternalInput")
    wname = ["wih0a", "wih0b", "whh0", "wih1", "whh1", "wih2", "whh2"]
    wdram = {n: nc.dram_tensor(n, [P, 512], mdt, kind="ExternalInput") for n in wname}
    bias_d = nc.dram_tensor("biases", [P, NL * 4], f32, kind="ExternalInput")
    hnT = nc.dram_tensor("hnT", [NL, P, np_pad], mdt, kind="ExternalOutput")
    cnT = nc.dram_tensor("cnT", [NL, P, np_pad], f32, kind="ExternalOutput")

    SIG = mybir.ActivationFunctionType.Sigmoid
    TANH = mybir.ActivationFunctionType.Tanh

    with tile.TileContext(nc) as tc:
        with (
            tc.tile_pool(name="const", bufs=1) as const_pool,
            tc.tile_pool(name="idx", bufs=1) as idx_pool,
            tc.tile_pool(name="stage", bufs=3) as stage_pool,
            tc.tile_pool(name="m", bufs=4) as m_pool,
            tc.tile_pool(name="small", bufs=6) as small_pool,
            tc.tile_pool(name="hagg", bufs=3) as hagg_pool,
            tc.tile_pool(name="xg", bufs=2) as xg_pool,
            tc.tile_pool(name="nodein", bufs=2) as nodein_pool,
            tc.tile_pool(name="gate", bufs=4) as gate_pool,
            tc.tile_pool(name="cell", bufs=2) as cell_pool,
            tc.tile_pool(name="hc", bufs=2) as hc_pool,
            tc.tile_pool(name="aggps", bufs=2, space="PSUM") as aggps_pool,
            tc.tile_pool(name="degps", bufs=2, space="PSUM") as degps_pool,
            tc.tile_pool(name="tps", bufs=2, space="PSUM") as tps_pool,
            tc.tile_pool(name="gps", bufs=2, space="PSUM") as gps_pool,
        ):
            # constants
            iota_i = const_pool.tile([P, P], mybir.dt.int32)
            nc.gpsimd.iota(iota_i[:], pattern=[[1, P]], base=0, channel_multiplier=0)
            iota_f = const_pool.tile([P, P], f32)
            nc.vector.tensor_copy(iota_f[:], iota_i[:])
            ident_f = const_pool.tile([P, P], f32)
            make_identity(nc, ident_f[:])
            ident = const_pool.tile([P, P], mdt)
            nc.vector.tensor_copy(ident[:], ident_f[:])
            ones_col = const_pool.tile([P, 1], f32)
            nc.vector.memset(ones_col[:], 1.0)
            w_sb = {}
            for n in wname:
                w_sb[n] = const_pool.tile([P, 512], mdt, tag=n, name=n)
                nc.sync.dma_start(w_sb[n][:], wdram[n][:])
            bias_sb = const_pool.tile([P, NL * 4], f32)
            nc.sync.dma_start(bias_sb[:], bias_d[:])
            idxs_sb = idx_pool.tile([P, nslot // 16], i16)
            nc.sync.dma_start(idxs_sb[:], idxs[:])
            slots_sb = idx_pool.tile([P, ncol], f32)
            nc.sync.dma_start(slots_sb[:], slots[:])
            recips_sb = idx_pool.tile([P, nblk], f32)
            nc.sync.dma_start(recips_sb[:], recips[:])

            for g in range(ngrp):
                b0 = g * bpg
                gc0 = b0 * cpb                       # first slot-col of group
                gcols = bpg * cpb
                ns = slice(g * 512, (g + 1) * 512)   # node range of group

                # ---- aggregation of the group's bpg blocks
                stage = stage_pool.tile([P, gcols, D], mdt)
                segs = []
                if b0 + bpg <= half_blks or b0 >= half_blks:
                    segs.append((b0, b0 + bpg, tab0 if b0 < half_blks else tab1))
                else:
                    segs.append((b0, half_blks, tab0))
                    segs.append((half_blks, b0 + bpg, tab1))
                for si, (sb, se, tab) in enumerate(segs):
                    c0_, c1_ = sb * cpb, se * cpb
                    nidx = (c1_ - c0_) * P
                    nc.gpsimd.dma_gather(
                        out_ap=stage[:, c0_ - gc0:c1_ - gc0, :],
                        in_ap=tab[:, :],
                        idxs_ap=idxs_sb[:, c0_ * 8:c1_ * 8],
                        num_idxs=nidx,
                        num_idxs_reg=nidx,
                        elem_size=D,
                        single_packet=False,
                        queue_num=(g + si) % nq,
                    )

                xg = xg_pool.tile([P, 512], mdt)     # h_agg feature-major
                for b in range(bpg):
                    blk = b0 + b
                    ps = aggps_pool.tile([P, 128], f32, space="PSUM")
                    dps = degps_pool.tile([P, 1], f32, space="PSUM")
                    for c in range(cpb):
                        col = blk * cpb + c
                        M = m_pool.tile([P, P], mdt)
                        nc.vector.tensor_tensor(
                            out=M[:],
                            in0=slots_sb[:, col:col + 1].to_broadcast([P, P])[:],
                            in1=iota_f[:],
                            op=mybir.AluOpType.is_equal,
                        )
                        nc.tensor.matmul(
                            out=ps[:], lhsT=M[:],
                            rhs=stage[:, col - gc0, :],
                            start=(c == 0), stop=(c == cpb - 1),
                        )
                        nc.tensor.matmul(
                            out=dps[:], lhsT=M[:].bitcast(f32), rhs=ones_col[:],
                            start=(c == 0), stop=(c == cpb - 1),
                        )
                    deg = small_pool.tile([P, 1], f32, tag="deg")
                    nc.vector.tensor_scalar_max(deg[:], dps[:], 1.0)
                    rec = small_pool.tile([P, 1], f32, tag="rec")
                    nc.vector.reciprocal(rec[:], deg[:])
                    hagg_nm = hagg_pool.tile([P, P], mdt)
                    nc.vector.tensor_scalar_mul(hagg_nm[:], ps[:], rec[:, 0:1])
                    tp = tps_pool.tile([P, P], mdt, space="PSUM")
                    nc.tensor.transpose(tp[:], hagg_nm[:], ident[:])
                    nc.scalar.copy(xg[:, b * P:(b + 1) * P], tp[:])

                # ---- LSTM, 3 layers over this group's 512 nodes
                featg = nodein_pool.tile([P, 512], mdt, tag="featg")
                nc.sync.dma_start(featg[:], featT[:, ns])
                hprev = None
                for l in range(NL):
                    h0g = nodein_pool.tile([P, 512], mdt, tag=f"h0g{l}")
                    nc.sync.dma_start(h0g[:], h0T[l, :, ns])
                    c0g = nodein_pool.tile([P, 512], f32, tag=f"c0g{l}")
                    nc.sync.dma_start(c0g[:], c0T[l, :, ns])

                    acts = []
                    for k in range(4):
                        gs = slice(k * 128, (k + 1) * 128)
                        ps = gps_pool.tile([P, 512], f32, space="PSUM")
                        if l == 0:
                            nc.tensor.matmul(ps[:], w_sb["wih0a"][:, gs],
                                             xg[:], start=True, stop=False)
                            nc.tensor.matmul(ps[:], w_sb["wih0b"][:, gs],
                                             featg[:], start=False, stop=False)
                            nc.tensor.matmul(ps[:], w_sb["whh0"][:, gs],
                                             h0g[:], start=False, stop=True)
                        else:
                            nc.tensor.matmul(ps[:], w_sb[f"wih{l}"][:, gs],
                                             hprev[:], start=True, stop=False)
                            nc.tensor.matmul(ps[:], w_sb[f"whh{l}"][:, gs],
                                             h0g[:], start=False, stop=True)
                        a = gate_pool.tile([P, 512], f32, tag=f"act{k}")
                        nc.scalar.activation(
                            a[:], ps[:], TANH if k == 2 else SIG,
                            bias=bias_sb[:, l * 4 + k:l * 4 + k + 1], scale=1.0)
                        acts.append(a)
                    i_t, f_t, g_t, o_t = acts
                    fc = cell_pool.tile([P, 512], f32, tag="fc")
                    nc.vector.tensor_tensor(fc[:], f_t[:], c0g[:],
                                            op=mybir.AluOpType.mult)
                    ig = cell_pool.tile([P, 512], f32, tag="ig")
                    nc.vector.tensor_tensor(ig[:], i_t[:], g_t[:],
                                            op=mybir.AluOpType.mult)
                    c_new = hc_pool.tile([P, 512], f32, tag=f"c{l}")
                    nc.vector.tensor_tensor(c_new[:], fc[:], ig[:],
                                            op=mybir.AluOpType.add)
                    tnh = cell_pool.tile([P, 512], f32, tag="tnh")
                    nc.scalar.activation(tnh[:], c_new[:], TANH)
                    h_new = hc_pool.tile([P, 512], mdt, tag=f"h{l}")
                    nc.vector.tensor_tensor(h_new[:], o_t[:], tnh[:],
                                            op=mybir.AluOpType.mult)
                    nc.sync.dma_start(hnT[l, :, ns], h_new[:])
                    nc.sync.dma_start(cnT[l, :, ns], c_new[:])
                    hprev = h_new
    nc.finalize()
    return nc


# ---------------------------------------------------------------- driver
def run(inputs, f32r=True, nq=4, scratch=32768, trace=False, tmpdir=None):
    from concourse.bass_utils import run_bass_kernel_spmd

    feature = np.asarray(inputs["feature"], np.float32)
    h0 = np.asarray(inputs["h0"], np.float32)
    c0 = np.asarray(inputs["c0"], np.float32)
    n_cores = 8
    cores, meta = preprocess(feature, inputs["src"], inputs["dst"], n_cores)
    nodes = node_arrays(feature, h0, c0, n_cores, meta["np_real"], meta["np_pad"])
    w = weight_arrays(inputs["w_ih0"], inputs["w_hh0"], inputs["w_ih_rest"],
                      inputs["w_hh_rest"], inputs["b_ih"], inputs["b_hh"])
    nc = build_program(meta, f32r=f32r, nq=nq, scratch=scratch)
    in_maps = [{**cores[m], **nodes[m], **w} for m in range(n_cores)]
    res = run_bass_kernel_spmd(nc, in_maps, core_ids=list(range(n_cores)),
                               trace=trace, tmpdir=tmpdir)

    N = feature.shape[0]
    np_real = meta["np_real"]
    h_n = np.empty((NL, N, D), np.float32)
    c_n = np.empty((NL, N, D), np.float32)
    for m in range(n_cores):
        sl = slice(m * np_real, (m + 1) * np_real)
        h_n[:, sl, :] = res.results[m]["hnT"][:, :, :np_real].transpose(0, 2, 1)
        c_n[:, sl, :] = res.results[m]["cnT"][:, :, :np_real].transpose(0, 2, 1)
    output = h_n[2][None].copy()
    return (output, h_n, c_n), res


# ---------------------------------------------------------------- entrypoint
def _ensure_profile_shim():
    """Register the axon NTFF profile hook if absent (harmless if unused)."""
    import sys, types
    if "antenv.axon_hooks" in sys.modules:
        return
    try:
        mod = types.ModuleType("antenv.axon_hooks")
        mod._hook = None
        mod.set_axon_ntff_profile_hook = lambda h: setattr(mod, "_hook", h)
        mod.get_axon_ntff_profile_hook = lambda: mod._hook
        sys.modules["antenv.axon_hooks"] = mod
        from trn_agent_boot.trn_boot import _ntff_profile_via_ctypes
        mod.set_axon_ntff_profile_hook(
            _ntff_profile_via_ctypes("/opt/axon/libaxon_pjrt.so"))
    except Exception:
        pass


def kernel(**inputs):
    """Full-input GCN-LSTM forward on 8 NeuronCores.

    Returns (output [1,N,128], h_n [3,N,128], c_n [3,N,128]) as np.float32,
    matching reference.reference().
    """
    _ensure_profile_shim()
    (output, h_n, c_n), _res = run(inputs, f32r=False)
    return output, h_n, c_n
